# revision 6
# baseline (speedup 1.0000x reference)
"""Trainium2 Bass kernel for nn_MixedGatedMLP (4-bit quantized gated MLP + LoRA).

v4: tensor-parallel over d_ff across 8 NeuronCores (F padded 11008->11264,
1408 rows/core).  The 16 codebook mask terms are merged on the PE array
(identity-matmul accumulation into PSUM, fp32) instead of DVE/GPSIMD adds.
 - codebook values baked as immediates (graph built after seeing inputs)
 - nibbles pre-split on host (u8 hi/lo planes); ACT converts u8->bf16;
   16 is_equal/mult mask terms on DVE; merge adds split DVE / dual-chain
   GPSIMD; blockwise absmax scale on DVE
 - down-proj dequant (to DRAM) interleaved with phase-1
 - phase-2 token-group-outer with all down weights resident, 8 PSUM banks,
   8 matmuls per LDWEIGHTS; ReduceScatter per 1024-token chunk, host
   reassembles the row interleave
"""

import sys

for _p in ("/opt/trn_rl_repo", "/root/.axon_site/_ro/trn_rl_repo"):
    if _p not in sys.path:
        sys.path.append(_p)

from contextlib import ExitStack

import numpy as np
import ml_dtypes

import concourse.bass as bass
import concourse.mybir as mybir
import concourse.tile as tile
from concourse import bacc
from concourse.bass_utils import run_bass_kernel_spmd

BF16 = ml_dtypes.bfloat16
NCORES = 8
ALU = mybir.AluOpType
AFT = mybir.ActivationFunctionType


class Cfg:
    def __init__(self, D=4096, T=4096, F=11008, R=16, block=64, ncores=8):
        self.D = D
        self.T = T
        self.F = F
        self.R = R
        self.block = block
        self.ncores = ncores
        unit = 2 * block * ncores
        self.FP = ((F + unit - 1) // unit) * unit   # 11264
        self.FS = self.FP // ncores                 # 1408
        self.TS = T // ncores                       # 512
        self.DP = D // 256                          # 16 byte-row chunks
        self.NT = T // 512                          # 8 token tiles
        self.f_slices = []
        f0 = 0
        while f0 < self.FS:
            w = min(512, self.FS - f0)
            self.f_slices.append((f0, w))
            f0 += w
        self.NTG = T // 128                         # 32 token groups
        self.NFG = self.FS // 128                   # 11 f groups
        self.ND = D // 512                          # 8 output-d slices
        self.TRS = 1024                             # tokens per ReduceScatter
        self.n_rs = T // self.TRS                   # 4
        self.j_chunks = []
        j0 = 0
        npairs = self.FS // 2
        while j0 < npairs:
            j1 = min(j0 + 128, npairs)
            self.j_chunks.append((j0, j1))
            j0 = j1
        # merge-term split across engines: DVE chain / GPSIMD chain / PE psum
        self.n_dve = 10
        self.n_gp = 4


def _dperm(D):
    """Row order of xT: per 256-d chunk, evens then odds."""
    idx = []
    for c in range(D // 256):
        base = 256 * c
        idx.extend(range(base, base + 256, 2))
        idx.extend(range(base + 1, base + 256, 2))
    return np.array(idx)


def _fperm_local(cfg):
    """Within-shard f order: per down j-chunk, even f (2j) then odd f (2j+1)."""
    idx = []
    for (j0, j1) in cfg.j_chunks:
        idx.extend(2 * j for j in range(j0, j1))
        idx.extend(2 * j + 1 for j in range(j0, j1))
    return np.array(idx)


def build_graph(cfg: Cfg, code_vals):
    """code_vals: 16 python floats (bf16-rounded codebook)."""
    nc = bacc.Bacc(None, num_devices=cfg.ncores)
    dt = mybir.dt
    D, T, FS, R = cfg.D, cfg.T, cfg.FS, cfg.R
    cv = [float(v) for v in code_vals]

    # ---- external inputs (per-core); nibble planes pre-split on host ----
    xT = nc.dram_tensor("xT", [D, T], dt.bfloat16, kind="ExternalInput")
    g_nib = nc.dram_tensor("g_nib", [D // 2, 2, FS], dt.uint8, kind="ExternalInput")
    u_nib = nc.dram_tensor("u_nib", [D // 2, 2, FS], dt.uint8, kind="ExternalInput")
    d_nib = nc.dram_tensor("d_nib", [FS // 2, 2, D], dt.uint8, kind="ExternalInput")
    g_am = nc.dram_tensor("g_am", [D // 2, FS], dt.bfloat16, kind="ExternalInput")
    u_am = nc.dram_tensor("u_am", [D // 2, FS], dt.bfloat16, kind="ExternalInput")
    d_am = nc.dram_tensor("d_am", [FS // 2, D], dt.bfloat16, kind="ExternalInput")
    a_gu = nc.dram_tensor("a_gu", [D, 2 * R], dt.bfloat16, kind="ExternalInput")
    b_g = nc.dram_tensor("b_g", [R, FS], dt.bfloat16, kind="ExternalInput")
    b_u = nc.dram_tensor("b_u", [R, FS], dt.bfloat16, kind="ExternalInput")
    a_d = nc.dram_tensor("a_d", [FS, R], dt.bfloat16, kind="ExternalInput")
    b_d = nc.dram_tensor("b_d", [R, D], dt.bfloat16, kind="ExternalInput")
    ident = nc.dram_tensor("ident", [128, 128], dt.bfloat16, kind="ExternalInput")

    # y_out rows: 4 blocks of 128 tokens (one per ReduceScatter chunk)
    y_out = nc.dram_tensor("y_out", [cfg.TS, D], dt.bfloat16, kind="ExternalOutput")

    # ---- internal DRAM ----
    x3_dram = nc.dram_tensor("x3_dram", [FS, T], dt.bfloat16, kind="Internal")
    xag_dram = nc.dram_tensor("xag_dram", [R, T], dt.bfloat16, kind="Internal")
    xau_dram = nc.dram_tensor("xau_dram", [R, T], dt.bfloat16, kind="Internal")
    x3a_dram = nc.dram_tensor("x3a_dram", [R, T], dt.bfloat16, kind="Internal")
    wd_dram = nc.dram_tensor("wd_dram", [FS, D], dt.bfloat16, kind="Internal")
    rs_in = [
        nc.dram_tensor(f"rs_in{c}", [cfg.TRS, D], dt.bfloat16, kind="Internal")
        for c in range(cfg.n_rs)
    ]
    rs_out = [
        nc.dram_tensor(f"rs_out{c}", [cfg.TRS // cfg.ncores, D], dt.bfloat16,
                       kind="Internal")
        for c in range(cfg.n_rs)
    ]

    rg = [list(range(cfg.ncores))]

    with tile.TileContext(nc) as tc, ExitStack() as ctx:
        const_pool = ctx.enter_context(tc.tile_pool(name="const", bufs=1))
        agu_sb = const_pool.tile([128, D // 128, 2 * R], dt.bfloat16)
        nc.sync.dma_start(agu_sb[:], a_gu.rearrange("(c p) r -> p c r", p=128))
        bg_sb = const_pool.tile([R, FS], dt.bfloat16)
        nc.sync.dma_start(bg_sb[:], b_g[:])
        bu_sb = const_pool.tile([R, FS], dt.bfloat16)
        nc.sync.dma_start(bu_sb[:], b_u[:])
        ad_sb = const_pool.tile([128, FS // 128, R], dt.bfloat16)
        nc.sync.dma_start(ad_sb[:], a_d.rearrange("(c p) r -> p c r", p=128))
        bd_sb = const_pool.tile([R, D], dt.bfloat16)
        nc.sync.dma_start(bd_sb[:], b_d[:])
        id_sb = const_pool.tile([128, 128], dt.bfloat16)
        nc.sync.dma_start(id_sb[:], ident[:])

        dq_pool = ctx.enter_context(tc.tile_pool(name="dq", bufs=3))
        tg_pool = ctx.enter_context(tc.tile_pool(name="tgp", bufs=4))
        tk_pool = ctx.enter_context(tc.tile_pool(name="tkp", bufs=4))
        wdd_pool = ctx.enter_context(tc.tile_pool(name="wdd", bufs=3))
        psm_pool = []

        def dequant_pair(nib_dram, am_dram, p0, pc, f0, fw, W, gp_terms,
                         planes=(0, 1), pbase=0):
            """Dequant nibble rows [p0:p0+pc] x cols [f0:f0+fw] into W
            (tile [128, 2, fw] bf16): W[:, 0, :] = hi plane, W[:, 1, :] = lo.
            The 16 mask terms are accumulated on the PE array (identity
            matmul) in fp32 PSUM.  Ops use partitions [pbase, pbase+pc)."""
            psl = slice(pbase, pbase + pc)
            U = dq_pool.tile([128, 2, fw], dt.uint8, tag="u")
            S2 = dq_pool.tile([128, 2, fw], dt.bfloat16, tag="sq")
            for pl in planes:
                nc.sync.dma_start(U[psl, pl],
                                  nib_dram[p0:p0 + pc, pl, f0:f0 + fw])
                nc.sync.dma_start(S2[psl, pl],
                                  am_dram[p0:p0 + pc, f0:f0 + fw])
            X = dq_pool.tile([128, 2, fw], dt.bfloat16, tag="x")
            if len(planes) == 2:
                s_ = (slice(None), slice(None))
            else:
                s_ = (planes[0], slice(None))
            nc.scalar.copy(X[(psl, *s_)], U[(psl, *s_)])
            Xs = X[(psl, *s_)]

            pm = psm_pool[0].tile([128, 2, 512], dt.float32, tag="pm",
                                 name="pm")
            n_dve, n_gp = cfg.n_dve, cfg.n_gp
            accD = dq_pool.tile([128, 2, fw], dt.bfloat16, tag="aD")
            accG = dq_pool.tile([128, 2, fw], dt.bfloat16, tag="aG")
            AD = accD[(psl, *s_)]
            AG = accG[(psl, *s_)]
            n_calls = (16 - n_dve - n_gp) + (1 if n_dve else 0) \
                + (1 if n_gp else 0)
            call_i = [0]

            def pe_merge(t):
                i = call_i[0]
                for pl in planes:
                    nc.tensor.matmul(pm[psl, pl, 0:fw], id_sb[psl, psl],
                                     t[psl, pl],
                                     start=(i == 0),
                                     stop=(i == n_calls - 1))
                call_i[0] = i + 1

            # DVE chain terms
            for j, k in enumerate(range(0, n_dve)):
                if j == 0:
                    nc.vector.tensor_scalar(AD, Xs, float(k), cv[k],
                                            ALU.is_equal, ALU.mult)
                else:
                    tk = tk_pool.tile([128, 2, fw], dt.bfloat16, tag="tk",
                                      name=f"tk{k}")
                    nc.vector.tensor_scalar(tk[(psl, *s_)], Xs, float(k),
                                            cv[k], ALU.is_equal, ALU.mult)
                    nc.vector.tensor_tensor(AD, AD, tk[(psl, *s_)], ALU.add)
            # GPSIMD chain terms
            for j, k in enumerate(range(n_dve, n_dve + n_gp)):
                if j == 0:
                    nc.vector.tensor_scalar(AG, Xs, float(k), cv[k],
                                            ALU.is_equal, ALU.mult)
                else:
                    tg = tg_pool.tile([128, 2, fw], dt.bfloat16, tag="tg",
                                      name=f"tg{k}")
                    nc.vector.tensor_scalar(tg[(psl, *s_)], Xs, float(k),
                                            cv[k], ALU.is_equal, ALU.mult)
                    nc.gpsimd.tensor_tensor(AG, AG, tg[(psl, *s_)], ALU.add)
            # PE-merged terms + chain heads
            for k in range(n_dve + n_gp, 16):
                tk = tk_pool.tile([128, 2, fw], dt.bfloat16, tag="tk",
                                  name=f"tk{k}")
                nc.vector.tensor_scalar(tk[(psl, *s_)], Xs, float(k), cv[k],
                                        ALU.is_equal, ALU.mult)
                pe_merge(tk)
            if n_dve:
                pe_merge(accD)
            if n_gp:
                pe_merge(accG)
            for pl in planes:
                nc.vector.tensor_tensor(W[psl, pl], pm[psl, pl, 0:fw],
                                        S2[psl, pl], ALU.mult)

        # down-proj dequant jobs (interleaved into phase 1), 512-wide
        down_jobs = []
        for dh in range(cfg.ND):
            dd0 = 512 * dh
            for ic, (j0, j1) in enumerate(cfg.j_chunks):
                down_jobs.append((ic, j0, j1 - j0, dd0))

        def emit_down(job):
            ic, j0, pc, dd0 = job
            Wt = wdd_pool.tile([128, 2, 512], dt.bfloat16, tag="wdd")
            if pc == 128:
                dequant_pair(d_nib, d_am, j0, pc, dd0, 512, Wt, 0)
                nc.sync.dma_start(
                    wd_dram[256 * ic:256 * ic + 128, dd0:dd0 + 512], Wt[:, 0])
                nc.sync.dma_start(
                    wd_dram[256 * ic + 128:256 * ic + 256, dd0:dd0 + 512],
                    Wt[:, 1])
            else:
                dequant_pair(d_nib, d_am, j0, pc, dd0, 512, Wt, 0,
                             planes=(0,), pbase=0)
                dequant_pair(d_nib, d_am, j0, pc, dd0, 512, Wt, 0,
                             planes=(1,), pbase=pc)
                nc.sync.dma_start(
                    wd_dram[256 * ic:256 * ic + pc, dd0:dd0 + 512],
                    Wt[0:pc, 0])
                nc.sync.dma_start(
                    wd_dram[256 * ic + pc:256 * ic + 2 * pc, dd0:dd0 + 512],
                    Wt[pc:2 * pc, 1])

        n_dj = len(down_jobs)
        dj_pos = 0

        # =============== phase 1: gate/up matmuls -> x3 ===============
        XQ = 4                       # xT chunk-quarters per token tile
        with (
            tc.tile_pool(name="w", bufs=cfg.DP + 2) as w_pool,
            tc.tile_pool(name="xt", bufs=XQ + 1) as xt_pool,
            tc.tile_pool(name="p1", bufs=3) as p1_pool,
            tc.tile_pool(name="psgu", bufs=2, space="PSUM") as ps_gu,
            tc.tile_pool(name="pslora", bufs=1, space="PSUM") as ps_lora,
            tc.tile_pool(name="psm", bufs=1, space="PSUM") as psm_pool_,
        ):
            psm_pool.append(psm_pool_)
            for si, (f0, fw) in enumerate(cfg.f_slices):
                wg = []
                wu = []
                for c in range(cfg.DP):
                    for name, bsrc, asrc, wlist in (
                        ("g", g_nib, g_am, wg), ("u", u_nib, u_am, wu),
                    ):
                        Wt = w_pool.tile([128, 2, fw], dt.bfloat16,
                                         tag=f"w{name}")
                        dequant_pair(bsrc, asrc, 128 * c, 128, f0, fw, Wt,
                                     0)
                        wlist.append(Wt)
                    if dj_pos < n_dj:
                        emit_down(down_jobs[dj_pos])
                        dj_pos += 1
                for t in range(cfg.NT):
                    tt = slice(512 * t, 512 * (t + 1))
                    nxq = 2 * cfg.DP // XQ
                    xts = []
                    for quar in range(XQ):
                        xt_t = xt_pool.tile([128, nxq, 512], dt.bfloat16,
                                            tag="xt")
                        for ci in range(nxq):
                            cc = nxq * quar + ci
                            nc.sync.dma_start(
                                xt_t[:, ci, :],
                                xT[128 * cc:128 * (cc + 1), tt])
                        xts.append(xt_t)

                    def xchunk(ci):
                        return xts[ci // nxq][:, ci % nxq, :]

                    if si == 0:
                        for ri, dst in ((0, xag_dram), (1, xau_dram)):
                            pa = ps_lora.tile([R, 512], dt.float32, tag="pa")
                            for ci in range(2 * cfg.DP):
                                nc.tensor.matmul(
                                    pa[:], agu_sb[:, ci, R * ri:R * (ri + 1)],
                                    xchunk(ci),
                                    start=(ci == 0),
                                    stop=(ci == 2 * cfg.DP - 1))
                            with tc.high_priority():
                                st = p1_pool.tile([R, 512], dt.bfloat16,
                                                  tag="st")
                                nc.scalar.copy(st[:], pa[:])
                                nc.sync.dma_start(dst[:, tt], st[:])
                    xag_t = p1_pool.tile([R, 512], dt.bfloat16, tag="xag_t")
                    nc.sync.dma_start(xag_t[:], xag_dram[:, tt])
                    xau_t = p1_pool.tile([R, 512], dt.bfloat16, tag="xau_t")
                    nc.sync.dma_start(xau_t[:], xau_dram[:, tt])
                    for g in range(fw // 128):
                        fg = slice(128 * g, 128 * (g + 1))
                        fga = slice(f0 + 128 * g, f0 + 128 * (g + 1))
                        pg = ps_gu.tile([128, 512], dt.float32, tag="pg")
                        pu = ps_gu.tile([128, 512], dt.float32, tag="pu")
                        for ci in range(2 * cfg.DP):
                            nc.tensor.matmul(pg[:], wg[ci // 2][:, ci % 2, fg],
                                             xchunk(ci),
                                             start=(ci == 0), stop=False)
                        nc.tensor.matmul(pg[:], bg_sb[:, fga], xag_t[:],
                                         start=False, stop=True)
                        for ci in range(2 * cfg.DP):
                            nc.tensor.matmul(pu[:], wu[ci // 2][:, ci % 2, fg],
                                             xchunk(ci),
                                             start=(ci == 0), stop=False)
                        nc.tensor.matmul(pu[:], bu_sb[:, fga], xau_t[:],
                                         start=False, stop=True)
                        with tc.high_priority():
                            sg = p1_pool.tile([128, 512], dt.bfloat16,
                                              tag="sg")
                            nc.scalar.activation(sg[:], pg[:], AFT.Silu)
                            pu_sb = p1_pool.tile([128, 512], dt.bfloat16,
                                                 tag="pusb")
                            nc.scalar.copy(pu_sb[:], pu[:])
                            x3t = p1_pool.tile([128, 512], dt.bfloat16,
                                               tag="x3t")
                            nc.vector.tensor_tensor(x3t[:], sg[:], pu_sb[:],
                                                    ALU.mult)
                            nc.sync.dma_start(x3_dram[fga, tt], x3t[:])
            while dj_pos < n_dj:
                emit_down(down_jobs[dj_pos])
                dj_pos += 1

        # =============== phase 2a: x3 @ Ad -> x3a ===============
        with (
            tc.tile_pool(name="p2a", bufs=4) as p2a_pool,
            tc.tile_pool(name="psda", bufs=1, space="PSUM") as ps_da,
        ):
            for t in range(cfg.NT):
                tt = slice(512 * t, 512 * (t + 1))
                pa = ps_da.tile([R, 512], dt.float32, tag="pa2")
                for g in range(cfg.NFG):
                    x3l = p2a_pool.tile([128, 512], dt.bfloat16, tag="x3a_in")
                    nc.sync.dma_start(x3l[:],
                                      x3_dram[128 * g:128 * (g + 1), tt])
                    nc.tensor.matmul(pa[:], ad_sb[:, g, :], x3l[:],
                                     start=(g == 0), stop=(g == cfg.NFG - 1))
                st2 = p2a_pool.tile([R, 512], dt.bfloat16, tag="st2")
                nc.scalar.copy(st2[:], pa[:])
                nc.sync.dma_start(x3a_dram[:, tt], st2[:])

        # ======== phase 2b: down matmul (tg-outer) + ReduceScatter ========
        with (
            tc.tile_pool(name="wdr", bufs=1) as wdr_pool,
            tc.tile_pool(name="x3p", bufs=2 * cfg.NFG + 2) as x3p_pool,
            tc.tile_pool(name="p2", bufs=6) as p2_pool,
            tc.tile_pool(name="psd", bufs=1, space="PSUM") as ps_d,
        ):
            wdq = []
            for g in range(cfg.NFG):
                wt = wdr_pool.tile([128, D], dt.bfloat16, tag=f"wdr{g}",
                                   name=f"wdr{g}")
                nc.sync.dma_start(wt[:], wd_dram[128 * g:128 * (g + 1), :])
                wdq.append(wt)
            for tg in range(cfg.NTG):
                tsl = slice(128 * tg, 128 * (tg + 1))
                rs_c = tg // 8
                row0 = 128 * (tg % 8)
                x3at = p2_pool.tile([R, 128], dt.bfloat16, tag="x3at")
                nc.sync.dma_start(x3at[:], x3a_dram[:, tsl])
                x3ls = []
                for g in range(cfg.NFG):
                    x3l = x3p_pool.tile([128, 128], dt.bfloat16, tag="x3l")
                    nc.sync.dma_start(x3l[:],
                                      x3_dram[128 * g:128 * (g + 1), tsl])
                    x3ls.append(x3l)
                for half in range(2):
                    dhs = range(4 * half, 4 * half + 4)
                    pds = {dh: ps_d.tile([128, 512], dt.float32,
                                         tag=f"pd{dh}",
                                         name=f"pd{tg}_{dh}")
                           for dh in dhs}
                    for g in range(cfg.NFG):
                        for dh in dhs:
                            nc.tensor.matmul(
                                pds[dh][:], x3ls[g][:],
                                wdq[g][:, 512 * dh:512 * (dh + 1)],
                                start=(g == 0), stop=False)
                    for dh in dhs:
                        nc.tensor.matmul(
                            pds[dh][:], x3at[:],
                            bd_sb[:, 512 * dh:512 * (dh + 1)],
                            start=False, stop=True)
                    with tc.high_priority():
                        for dh in dhs:
                            yb = p2_pool.tile([128, 512], dt.bfloat16,
                                              tag="yb")
                            nc.scalar.copy(yb[:], pds[dh][:])
                            nc.sync.dma_start(
                                rs_in[rs_c][row0:row0 + 128,
                                            512 * dh:512 * (dh + 1)], yb[:])
                if tg % 8 == 7:
                    nc.gpsimd.collective_compute(
                        "ReduceScatter", ALU.add, replica_groups=rg,
                        ins=[rs_in[rs_c][:, :].opt()],
                        outs=[rs_out[rs_c][:, :].opt()],
                    )
                    nc.sync.dma_start(
                        y_out[128 * rs_c:128 * (rs_c + 1), :],
                        rs_out[rs_c][:, :])

    nc.compile()
    return nc


# ----------------- host side -----------------

_CACHE = {}


def _get_graph(cfg: Cfg, code_vals):
    key = (cfg.D, cfg.T, cfg.F, cfg.ncores, tuple(code_vals))
    if key not in _CACHE:
        _CACHE[key] = build_graph(cfg, code_vals)
    return _CACHE[key]


def _prep_inputs(cfg: Cfg, inputs):
    """Shard + lay out the full inputs for each core. Marshalling only."""
    D, T, F, FP, FS, R = cfg.D, cfg.T, cfg.F, cfg.FP, cfg.FS, cfg.R
    blk = cfg.block
    dperm = _dperm(D)
    fperm = _fperm_local(cfg)

    x = inputs["x"]
    xT = np.ascontiguousarray(x.T[dperm]).astype(BF16)

    def split_nib(bT):
        """bytes [P, Q] -> [P, 2, Q] u8 nibble planes (hi, lo)."""
        return np.ascontiguousarray(
            np.stack([bT >> 4, bT & 15], axis=1))

    def pack_rows(packed, absmax):
        """gate/up: packed [F*D/2] -> per-core (nib [D/2, 2, FS], am)."""
        b = (packed.astype(np.int64) & 0xFF).astype(np.uint8).reshape(F, D // 2)
        b = np.concatenate([b, np.zeros((FP - F, D // 2), np.uint8)], 0)
        am = absmax.reshape(F, D // blk).astype(np.float32)
        am = np.concatenate([am, np.zeros((FP - F, D // blk), np.float32)], 0)
        outs = []
        for i in range(cfg.ncores):
            bs = b[FS * i:FS * (i + 1)][fperm]           # [FS, D/2]
            ams = am[FS * i:FS * (i + 1)][fperm]         # [FS, D/blk]
            bT = np.ascontiguousarray(bs.T)              # [D/2, FS]
            amT = np.repeat(ams.T.astype(BF16), blk // 2, axis=0)  # [D/2, FS]
            outs.append((split_nib(bT), np.ascontiguousarray(amT)))
        return outs

    def pack_down(packed, absmax):
        """down: packed [D*F/2] -> per-core (nib [FS/2, 2, D], am [FS/2, D])."""
        b = (packed.astype(np.int64) & 0xFF).astype(np.uint8).reshape(D, F // 2)
        b = np.concatenate([b, np.zeros((D, (FP - F) // 2), np.uint8)], 1)
        am = absmax.reshape(D, F // blk).astype(np.float32)
        am = np.concatenate([am, np.zeros((D, (FP - F) // blk), np.float32)], 1)
        outs = []
        npairs = FS // 2
        nblk = FS // blk
        for i in range(cfg.ncores):
            bs = b[:, npairs * i:npairs * (i + 1)]       # [D, FS/2]
            ams = am[:, nblk * i:nblk * (i + 1)]         # [D, FS/blk]
            bT = np.ascontiguousarray(bs.T)              # [FS/2, D]
            amT = np.repeat(ams.T.astype(BF16), blk // 2, axis=0)  # [FS/2, D]
            outs.append((split_nib(bT), np.ascontiguousarray(amT)))
        return outs

    g = pack_rows(inputs["w_gate_packed"], inputs["w_gate_absmax"])
    u = pack_rows(inputs["w_up_packed"], inputs["w_up_absmax"])
    d = pack_down(inputs["w_down_packed"], inputs["w_down_absmax"])

    a_gu = np.concatenate(
        [inputs["w_gate_lora_a"], inputs["w_up_lora_a"]], axis=1
    )[dperm].astype(BF16)

    def pad_cols(m):
        return np.concatenate([m, np.zeros((m.shape[0], FP - F), m.dtype)], 1)

    b_g_full = pad_cols(inputs["w_gate_lora_b"].astype(np.float32))
    b_u_full = pad_cols(inputs["w_up_lora_b"].astype(np.float32))
    a_d_full = np.concatenate(
        [inputs["w_down_lora_a"].astype(np.float32),
         np.zeros((FP - F, R), np.float32)], 0
    )
    b_d = inputs["w_down_lora_b"].astype(BF16)

    in_maps = []
    for i in range(cfg.ncores):
        fsl = slice(FS * i, FS * (i + 1))
        in_maps.append({
            "xT": xT,
            "g_nib": g[i][0], "g_am": g[i][1],
            "u_nib": u[i][0], "u_am": u[i][1],
            "d_nib": d[i][0], "d_am": d[i][1],
            "a_gu": a_gu,
            "b_g": np.ascontiguousarray(b_g_full[:, fsl][:, fperm]).astype(BF16),
            "b_u": np.ascontiguousarray(b_u_full[:, fsl][:, fperm]).astype(BF16),
            "a_d": np.ascontiguousarray(a_d_full[fsl][fperm]).astype(BF16),
            "b_d": b_d,
            "ident": np.eye(128, dtype=BF16),
        })
    return in_maps


def _code_vals(inputs):
    # bf16-rounded codebook values as python floats
    return [float(v) for v in
            np.asarray(inputs["code"]).astype(BF16).astype(np.float32)]


def _assemble(cfg: Cfg, res):
    """Undo the ReduceScatter row interleave: core r's y_out row block c
    holds tokens [TRS*c + 128*r, +128)."""
    T, D = cfg.T, cfg.D
    y = np.empty((T, D), dtype=np.float32)
    for r in range(cfg.ncores):
        yr = res.results[r]["y_out"].astype(np.float32)
        for c in range(cfg.n_rs):
            t0 = cfg.TRS * c + 128 * r
            y[t0:t0 + 128] = yr[128 * c:128 * (c + 1)]
    return y


def run(cfg: Cfg, inputs, trace=False, **kwargs):
    code_vals = _code_vals(inputs)
    nc = _get_graph(cfg, code_vals)
    in_maps = _prep_inputs(cfg, inputs)
    res = run_bass_kernel_spmd(
        nc, in_maps, core_ids=list(range(cfg.ncores)), trace=trace, **kwargs
    )
    return _assemble(cfg, res), res


def kernel(**inputs) -> np.ndarray:
    cfg = Cfg()
    y, _ = run(cfg, inputs)
    return y.astype(np.float32)


# revision 7
# speedup vs baseline: 1.1630x; 1.1630x over previous
"""Trainium2 Bass kernel for nn_MixedGatedMLP (4-bit quantized gated MLP + LoRA).

v4: tensor-parallel over d_ff across 8 NeuronCores (F padded 11008->11264,
1408 rows/core).  The 16 codebook mask terms are merged on the PE array
(identity-matmul accumulation into PSUM, fp32) instead of DVE/GPSIMD adds.
 - codebook values baked as immediates (graph built after seeing inputs)
 - nibbles pre-split on host (u8 hi/lo planes); ACT converts u8->bf16;
   16 is_equal/mult mask terms on DVE; merge adds split DVE / dual-chain
   GPSIMD; blockwise absmax scale on DVE
 - down-proj dequant (to DRAM) interleaved with phase-1
 - phase-2 token-group-outer with all down weights resident, 8 PSUM banks,
   8 matmuls per LDWEIGHTS; ReduceScatter per 1024-token chunk, host
   reassembles the row interleave
"""

import sys

for _p in ("/opt/trn_rl_repo", "/root/.axon_site/_ro/trn_rl_repo"):
    if _p not in sys.path:
        sys.path.append(_p)

from contextlib import ExitStack

import numpy as np
import ml_dtypes

import concourse.bass as bass
import concourse.mybir as mybir
import concourse.tile as tile
from concourse import bacc
from concourse.bass_utils import run_bass_kernel_spmd

BF16 = ml_dtypes.bfloat16
NCORES = 8
ALU = mybir.AluOpType
AFT = mybir.ActivationFunctionType


class Cfg:
    def __init__(self, D=4096, T=4096, F=11008, R=16, block=64, ncores=8):
        self.D = D
        self.T = T
        self.F = F
        self.R = R
        self.block = block
        self.ncores = ncores
        unit = 2 * block * ncores
        self.FP = ((F + unit - 1) // unit) * unit   # 11264
        self.FS = self.FP // ncores                 # 1408
        self.TS = T // ncores                       # 512
        self.DP = D // 256                          # 16 byte-row chunks
        self.NT = T // 512                          # 8 token tiles
        self.f_slices = []
        f0 = 0
        while f0 < self.FS:
            w = min(512, self.FS - f0)
            self.f_slices.append((f0, w))
            f0 += w
        self.NTG = T // 128                         # 32 token groups
        self.NFG = self.FS // 128                   # 11 f groups
        self.ND = D // 512                          # 8 output-d slices
        self.TRS = 1024                             # tokens per ReduceScatter
        self.n_rs = T // self.TRS                   # 4
        self.j_chunks = []
        j0 = 0
        npairs = self.FS // 2
        while j0 < npairs:
            j1 = min(j0 + 128, npairs)
            self.j_chunks.append((j0, j1))
            j0 = j1
        # merge-term split across engines: DVE chain / GPSIMD chain / PE psum
        self.n_dve = 4
        self.n_gp = 3


def _dperm(D):
    """Row order of xT: per 256-d chunk, evens then odds."""
    idx = []
    for c in range(D // 256):
        base = 256 * c
        idx.extend(range(base, base + 256, 2))
        idx.extend(range(base + 1, base + 256, 2))
    return np.array(idx)


def _fperm_local(cfg):
    """Within-shard f order: per down j-chunk, even f (2j) then odd f (2j+1)."""
    idx = []
    for (j0, j1) in cfg.j_chunks:
        idx.extend(2 * j for j in range(j0, j1))
        idx.extend(2 * j + 1 for j in range(j0, j1))
    return np.array(idx)


def build_graph(cfg: Cfg, code_vals):
    """code_vals: 16 python floats (bf16-rounded codebook)."""
    nc = bacc.Bacc(None, num_devices=cfg.ncores)
    dt = mybir.dt
    D, T, FS, R = cfg.D, cfg.T, cfg.FS, cfg.R
    cv = [float(v) for v in code_vals]

    # ---- external inputs (per-core); nibble planes pre-split on host ----
    xT = nc.dram_tensor("xT", [D, T], dt.bfloat16, kind="ExternalInput")
    g_nib = nc.dram_tensor("g_nib", [D // 2, 2, FS], dt.uint8, kind="ExternalInput")
    u_nib = nc.dram_tensor("u_nib", [D // 2, 2, FS], dt.uint8, kind="ExternalInput")
    d_nib = nc.dram_tensor("d_nib", [FS // 2, 2, D], dt.uint8, kind="ExternalInput")
    g_am = nc.dram_tensor("g_am", [D // 2, FS], dt.bfloat16, kind="ExternalInput")
    u_am = nc.dram_tensor("u_am", [D // 2, FS], dt.bfloat16, kind="ExternalInput")
    d_am = nc.dram_tensor("d_am", [FS // 2, D], dt.bfloat16, kind="ExternalInput")
    a_gu = nc.dram_tensor("a_gu", [D, 2 * R], dt.bfloat16, kind="ExternalInput")
    b_g = nc.dram_tensor("b_g", [R, FS], dt.bfloat16, kind="ExternalInput")
    b_u = nc.dram_tensor("b_u", [R, FS], dt.bfloat16, kind="ExternalInput")
    a_d = nc.dram_tensor("a_d", [FS, R], dt.bfloat16, kind="ExternalInput")
    b_d = nc.dram_tensor("b_d", [R, D], dt.bfloat16, kind="ExternalInput")
    ident = nc.dram_tensor("ident", [128, 128], dt.bfloat16, kind="ExternalInput")

    # y_out rows: 4 blocks of 128 tokens (one per ReduceScatter chunk)
    y_out = nc.dram_tensor("y_out", [cfg.TS, D], dt.bfloat16, kind="ExternalOutput")

    # ---- internal DRAM ----
    x3_dram = nc.dram_tensor("x3_dram", [FS, T], dt.bfloat16, kind="Internal")
    xag_dram = nc.dram_tensor("xag_dram", [R, T], dt.bfloat16, kind="Internal")
    xau_dram = nc.dram_tensor("xau_dram", [R, T], dt.bfloat16, kind="Internal")
    x3a_dram = nc.dram_tensor("x3a_dram", [R, T], dt.bfloat16, kind="Internal")
    wd_dram = nc.dram_tensor("wd_dram", [FS, D], dt.bfloat16, kind="Internal")
    rs_in = [
        nc.dram_tensor(f"rs_in{c}", [cfg.TRS, D], dt.bfloat16, kind="Internal")
        for c in range(cfg.n_rs)
    ]
    rs_out = [
        nc.dram_tensor(f"rs_out{c}", [cfg.TRS // cfg.ncores, D], dt.bfloat16,
                       kind="Internal")
        for c in range(cfg.n_rs)
    ]

    rg = [list(range(cfg.ncores))]

    with tile.TileContext(nc) as tc, ExitStack() as ctx:
        const_pool = ctx.enter_context(tc.tile_pool(name="const", bufs=1))
        agu_sb = const_pool.tile([128, D // 128, 2 * R], dt.bfloat16)
        nc.sync.dma_start(agu_sb[:], a_gu.rearrange("(c p) r -> p c r", p=128))
        bg_sb = const_pool.tile([R, FS], dt.bfloat16)
        nc.sync.dma_start(bg_sb[:], b_g[:])
        bu_sb = const_pool.tile([R, FS], dt.bfloat16)
        nc.sync.dma_start(bu_sb[:], b_u[:])
        ad_sb = const_pool.tile([128, FS // 128, R], dt.bfloat16)
        nc.sync.dma_start(ad_sb[:], a_d.rearrange("(c p) r -> p c r", p=128))
        bd_sb = const_pool.tile([R, D], dt.bfloat16)
        nc.sync.dma_start(bd_sb[:], b_d[:])
        id_sb = const_pool.tile([128, 128], dt.bfloat16)
        nc.sync.dma_start(id_sb[:], ident[:])

        dq_pool = ctx.enter_context(tc.tile_pool(name="dq", bufs=3))
        tg_pool = ctx.enter_context(tc.tile_pool(name="tgp", bufs=4))
        tk_pool = ctx.enter_context(tc.tile_pool(name="tkp", bufs=4))
        wdd_pool = ctx.enter_context(tc.tile_pool(name="wdd", bufs=3))
        psm_pool = []

        def dequant_pair(nib_dram, am_dram, p0, pc, f0, fw, W, gp_terms,
                         planes=(0, 1), pbase=0):
            """Dequant nibble rows [p0:p0+pc] x cols [f0:f0+fw] into W
            (tile [128, 2, fw] bf16): W[:, 0, :] = hi plane, W[:, 1, :] = lo.
            The 16 mask terms are accumulated on the PE array (identity
            matmul) in fp32 PSUM.  Ops use partitions [pbase, pbase+pc)."""
            psl = slice(pbase, pbase + pc)
            U = dq_pool.tile([128, 2, fw], dt.uint8, tag="u")
            S2 = dq_pool.tile([128, 2, fw], dt.bfloat16, tag="sq")
            for pl in planes:
                nc.sync.dma_start(U[psl, pl],
                                  nib_dram[p0:p0 + pc, pl, f0:f0 + fw])
                nc.sync.dma_start(S2[psl, pl],
                                  am_dram[p0:p0 + pc, f0:f0 + fw])
            X = dq_pool.tile([128, 2, fw], dt.bfloat16, tag="x")
            if len(planes) == 2:
                s_ = (slice(None), slice(None))
            else:
                s_ = (planes[0], slice(None))
            nc.scalar.copy(X[(psl, *s_)], U[(psl, *s_)])
            Xs = X[(psl, *s_)]

            pm = psm_pool[0].tile([128, 2, 512], dt.float32, tag="pm",
                                 name="pm")
            n_dve, n_gp = cfg.n_dve, cfg.n_gp
            accD = dq_pool.tile([128, 2, fw], dt.bfloat16, tag="aD")
            accG = dq_pool.tile([128, 2, fw], dt.bfloat16, tag="aG")
            AD = accD[(psl, *s_)]
            AG = accG[(psl, *s_)]
            n_calls = (16 - n_dve - n_gp) + (1 if n_dve else 0) \
                + (1 if n_gp else 0)
            call_i = [0]

            def pe_merge(t):
                i = call_i[0]
                for pl in planes:
                    nc.tensor.matmul(pm[psl, pl, 0:fw], id_sb[psl, psl],
                                     t[psl, pl],
                                     start=(i == 0),
                                     stop=(i == n_calls - 1))
                call_i[0] = i + 1

            # DVE chain terms
            for j, k in enumerate(range(0, n_dve)):
                if j == 0:
                    nc.vector.tensor_scalar(AD, Xs, float(k), cv[k],
                                            ALU.is_equal, ALU.mult)
                else:
                    tk = tk_pool.tile([128, 2, fw], dt.bfloat16, tag="tk",
                                      name=f"tk{k}")
                    nc.vector.tensor_scalar(tk[(psl, *s_)], Xs, float(k),
                                            cv[k], ALU.is_equal, ALU.mult)
                    nc.vector.tensor_tensor(AD, AD, tk[(psl, *s_)], ALU.add)
            # GPSIMD chain terms
            for j, k in enumerate(range(n_dve, n_dve + n_gp)):
                if j == 0:
                    nc.vector.tensor_scalar(AG, Xs, float(k), cv[k],
                                            ALU.is_equal, ALU.mult)
                else:
                    tg = tg_pool.tile([128, 2, fw], dt.bfloat16, tag="tg",
                                      name=f"tg{k}")
                    nc.vector.tensor_scalar(tg[(psl, *s_)], Xs, float(k),
                                            cv[k], ALU.is_equal, ALU.mult)
                    nc.gpsimd.tensor_tensor(AG, AG, tg[(psl, *s_)], ALU.add)
            # PE-merged terms + chain heads
            for k in range(n_dve + n_gp, 16):
                tk = tk_pool.tile([128, 2, fw], dt.bfloat16, tag="tk",
                                  name=f"tk{k}")
                nc.vector.tensor_scalar(tk[(psl, *s_)], Xs, float(k), cv[k],
                                        ALU.is_equal, ALU.mult)
                pe_merge(tk)
            if n_dve:
                pe_merge(accD)
            if n_gp:
                pe_merge(accG)
            for pl in planes:
                nc.vector.tensor_tensor(W[psl, pl], pm[psl, pl, 0:fw],
                                        S2[psl, pl], ALU.mult)

        # down-proj dequant jobs (interleaved into phase 1), 512-wide
        down_jobs = []
        for dh in range(cfg.ND):
            dd0 = 512 * dh
            for ic, (j0, j1) in enumerate(cfg.j_chunks):
                down_jobs.append((ic, j0, j1 - j0, dd0))

        def emit_down(job):
            ic, j0, pc, dd0 = job
            Wt = wdd_pool.tile([128, 2, 512], dt.bfloat16, tag="wdd")
            if pc == 128:
                dequant_pair(d_nib, d_am, j0, pc, dd0, 512, Wt, 0)
                nc.sync.dma_start(
                    wd_dram[256 * ic:256 * ic + 128, dd0:dd0 + 512], Wt[:, 0])
                nc.sync.dma_start(
                    wd_dram[256 * ic + 128:256 * ic + 256, dd0:dd0 + 512],
                    Wt[:, 1])
            else:
                dequant_pair(d_nib, d_am, j0, pc, dd0, 512, Wt, 0,
                             planes=(0,), pbase=0)
                dequant_pair(d_nib, d_am, j0, pc, dd0, 512, Wt, 0,
                             planes=(1,), pbase=pc)
                nc.sync.dma_start(
                    wd_dram[256 * ic:256 * ic + pc, dd0:dd0 + 512],
                    Wt[0:pc, 0])
                nc.sync.dma_start(
                    wd_dram[256 * ic + pc:256 * ic + 2 * pc, dd0:dd0 + 512],
                    Wt[pc:2 * pc, 1])

        n_dj = len(down_jobs)
        dj_pos = 0

        # =============== phase 1: gate/up matmuls -> x3 ===============
        XQ = 4                       # xT chunk-quarters per token tile
        with (
            tc.tile_pool(name="w", bufs=cfg.DP + 2) as w_pool,
            tc.tile_pool(name="xt", bufs=XQ + 1) as xt_pool,
            tc.tile_pool(name="p1", bufs=3) as p1_pool,
            tc.tile_pool(name="psgu", bufs=2, space="PSUM") as ps_gu,
            tc.tile_pool(name="pslora", bufs=1, space="PSUM") as ps_lora,
            tc.tile_pool(name="psm", bufs=1, space="PSUM") as psm_pool_,
        ):
            psm_pool.append(psm_pool_)
            for si, (f0, fw) in enumerate(cfg.f_slices):
                wg = []
                wu = []
                for c in range(cfg.DP):
                    for name, bsrc, asrc, wlist in (
                        ("g", g_nib, g_am, wg), ("u", u_nib, u_am, wu),
                    ):
                        Wt = w_pool.tile([128, 2, fw], dt.bfloat16,
                                         tag=f"w{name}")
                        dequant_pair(bsrc, asrc, 128 * c, 128, f0, fw, Wt,
                                     0)
                        wlist.append(Wt)
                    if dj_pos < n_dj:
                        emit_down(down_jobs[dj_pos])
                        dj_pos += 1
                for t in range(cfg.NT):
                    tt = slice(512 * t, 512 * (t + 1))
                    nxq = 2 * cfg.DP // XQ
                    xts = []
                    for quar in range(XQ):
                        xt_t = xt_pool.tile([128, nxq, 512], dt.bfloat16,
                                            tag="xt")
                        for ci in range(nxq):
                            cc = nxq * quar + ci
                            nc.sync.dma_start(
                                xt_t[:, ci, :],
                                xT[128 * cc:128 * (cc + 1), tt])
                        xts.append(xt_t)

                    def xchunk(ci):
                        return xts[ci // nxq][:, ci % nxq, :]

                    if si == 0:
                        for ri, dst in ((0, xag_dram), (1, xau_dram)):
                            pa = ps_lora.tile([R, 512], dt.float32, tag="pa")
                            for ci in range(2 * cfg.DP):
                                nc.tensor.matmul(
                                    pa[:], agu_sb[:, ci, R * ri:R * (ri + 1)],
                                    xchunk(ci),
                                    start=(ci == 0),
                                    stop=(ci == 2 * cfg.DP - 1))
                            with tc.high_priority():
                                st = p1_pool.tile([R, 512], dt.bfloat16,
                                                  tag="st")
                                nc.scalar.copy(st[:], pa[:])
                                nc.sync.dma_start(dst[:, tt], st[:])
                    xag_t = p1_pool.tile([R, 512], dt.bfloat16, tag="xag_t")
                    nc.sync.dma_start(xag_t[:], xag_dram[:, tt])
                    xau_t = p1_pool.tile([R, 512], dt.bfloat16, tag="xau_t")
                    nc.sync.dma_start(xau_t[:], xau_dram[:, tt])
                    for g in range(fw // 128):
                        fg = slice(128 * g, 128 * (g + 1))
                        fga = slice(f0 + 128 * g, f0 + 128 * (g + 1))
                        pg = ps_gu.tile([128, 512], dt.float32, tag="pg")
                        pu = ps_gu.tile([128, 512], dt.float32, tag="pu")
                        for ci in range(2 * cfg.DP):
                            nc.tensor.matmul(pg[:], wg[ci // 2][:, ci % 2, fg],
                                             xchunk(ci),
                                             start=(ci == 0), stop=False)
                        nc.tensor.matmul(pg[:], bg_sb[:, fga], xag_t[:],
                                         start=False, stop=True)
                        for ci in range(2 * cfg.DP):
                            nc.tensor.matmul(pu[:], wu[ci // 2][:, ci % 2, fg],
                                             xchunk(ci),
                                             start=(ci == 0), stop=False)
                        nc.tensor.matmul(pu[:], bu_sb[:, fga], xau_t[:],
                                         start=False, stop=True)
                        with tc.high_priority():
                            sg = p1_pool.tile([128, 512], dt.bfloat16,
                                              tag="sg")
                            nc.scalar.activation(sg[:], pg[:], AFT.Silu)
                            pu_sb = p1_pool.tile([128, 512], dt.bfloat16,
                                                 tag="pusb")
                            nc.scalar.copy(pu_sb[:], pu[:])
                            x3t = p1_pool.tile([128, 512], dt.bfloat16,
                                               tag="x3t")
                            nc.vector.tensor_tensor(x3t[:], sg[:], pu_sb[:],
                                                    ALU.mult)
                            nc.sync.dma_start(x3_dram[fga, tt], x3t[:])
            while dj_pos < n_dj:
                emit_down(down_jobs[dj_pos])
                dj_pos += 1

        # =============== phase 2a: x3 @ Ad -> x3a ===============
        with (
            tc.tile_pool(name="p2a", bufs=4) as p2a_pool,
            tc.tile_pool(name="psda", bufs=1, space="PSUM") as ps_da,
        ):
            for t in range(cfg.NT):
                tt = slice(512 * t, 512 * (t + 1))
                pa = ps_da.tile([R, 512], dt.float32, tag="pa2")
                for g in range(cfg.NFG):
                    x3l = p2a_pool.tile([128, 512], dt.bfloat16, tag="x3a_in")
                    nc.sync.dma_start(x3l[:],
                                      x3_dram[128 * g:128 * (g + 1), tt])
                    nc.tensor.matmul(pa[:], ad_sb[:, g, :], x3l[:],
                                     start=(g == 0), stop=(g == cfg.NFG - 1))
                st2 = p2a_pool.tile([R, 512], dt.bfloat16, tag="st2")
                nc.scalar.copy(st2[:], pa[:])
                nc.sync.dma_start(x3a_dram[:, tt], st2[:])

        # ======== phase 2b: down matmul (tg-outer) + ReduceScatter ========
        with (
            tc.tile_pool(name="wdr", bufs=1) as wdr_pool,
            tc.tile_pool(name="x3p", bufs=2 * cfg.NFG + 2) as x3p_pool,
            tc.tile_pool(name="p2", bufs=6) as p2_pool,
            tc.tile_pool(name="psd", bufs=1, space="PSUM") as ps_d,
        ):
            wdq = []
            for g in range(cfg.NFG):
                wt = wdr_pool.tile([128, D], dt.bfloat16, tag=f"wdr{g}",
                                   name=f"wdr{g}")
                nc.sync.dma_start(wt[:], wd_dram[128 * g:128 * (g + 1), :])
                wdq.append(wt)
            for tg in range(cfg.NTG):
                tsl = slice(128 * tg, 128 * (tg + 1))
                rs_c = tg // 8
                row0 = 128 * (tg % 8)
                x3at = p2_pool.tile([R, 128], dt.bfloat16, tag="x3at")
                nc.sync.dma_start(x3at[:], x3a_dram[:, tsl])
                x3ls = []
                for g in range(cfg.NFG):
                    x3l = x3p_pool.tile([128, 128], dt.bfloat16, tag="x3l")
                    nc.sync.dma_start(x3l[:],
                                      x3_dram[128 * g:128 * (g + 1), tsl])
                    x3ls.append(x3l)
                for half in range(2):
                    dhs = range(4 * half, 4 * half + 4)
                    pds = {dh: ps_d.tile([128, 512], dt.float32,
                                         tag=f"pd{dh}",
                                         name=f"pd{tg}_{dh}")
                           for dh in dhs}
                    for g in range(cfg.NFG):
                        for dh in dhs:
                            nc.tensor.matmul(
                                pds[dh][:], x3ls[g][:],
                                wdq[g][:, 512 * dh:512 * (dh + 1)],
                                start=(g == 0), stop=False)
                    for dh in dhs:
                        nc.tensor.matmul(
                            pds[dh][:], x3at[:],
                            bd_sb[:, 512 * dh:512 * (dh + 1)],
                            start=False, stop=True)
                    with tc.high_priority():
                        for dh in dhs:
                            yb = p2_pool.tile([128, 512], dt.bfloat16,
                                              tag="yb")
                            nc.scalar.copy(yb[:], pds[dh][:])
                            nc.sync.dma_start(
                                rs_in[rs_c][row0:row0 + 128,
                                            512 * dh:512 * (dh + 1)], yb[:])
                if tg % 8 == 7:
                    nc.gpsimd.collective_compute(
                        "ReduceScatter", ALU.add, replica_groups=rg,
                        ins=[rs_in[rs_c][:, :].opt()],
                        outs=[rs_out[rs_c][:, :].opt()],
                    )
                    nc.sync.dma_start(
                        y_out[128 * rs_c:128 * (rs_c + 1), :],
                        rs_out[rs_c][:, :])

    nc.compile()
    return nc


# ----------------- host side -----------------

_CACHE = {}


def _get_graph(cfg: Cfg, code_vals):
    key = (cfg.D, cfg.T, cfg.F, cfg.ncores, tuple(code_vals))
    if key not in _CACHE:
        _CACHE[key] = build_graph(cfg, code_vals)
    return _CACHE[key]


def _prep_inputs(cfg: Cfg, inputs):
    """Shard + lay out the full inputs for each core. Marshalling only."""
    D, T, F, FP, FS, R = cfg.D, cfg.T, cfg.F, cfg.FP, cfg.FS, cfg.R
    blk = cfg.block
    dperm = _dperm(D)
    fperm = _fperm_local(cfg)

    x = inputs["x"]
    xT = np.ascontiguousarray(x.T[dperm]).astype(BF16)

    def split_nib(bT):
        """bytes [P, Q] -> [P, 2, Q] u8 nibble planes (hi, lo)."""
        return np.ascontiguousarray(
            np.stack([bT >> 4, bT & 15], axis=1))

    def pack_rows(packed, absmax):
        """gate/up: packed [F*D/2] -> per-core (nib [D/2, 2, FS], am)."""
        b = (packed.astype(np.int64) & 0xFF).astype(np.uint8).reshape(F, D // 2)
        b = np.concatenate([b, np.zeros((FP - F, D // 2), np.uint8)], 0)
        am = absmax.reshape(F, D // blk).astype(np.float32)
        am = np.concatenate([am, np.zeros((FP - F, D // blk), np.float32)], 0)
        outs = []
        for i in range(cfg.ncores):
            bs = b[FS * i:FS * (i + 1)][fperm]           # [FS, D/2]
            ams = am[FS * i:FS * (i + 1)][fperm]         # [FS, D/blk]
            bT = np.ascontiguousarray(bs.T)              # [D/2, FS]
            amT = np.repeat(ams.T.astype(BF16), blk // 2, axis=0)  # [D/2, FS]
            outs.append((split_nib(bT), np.ascontiguousarray(amT)))
        return outs

    def pack_down(packed, absmax):
        """down: packed [D*F/2] -> per-core (nib [FS/2, 2, D], am [FS/2, D])."""
        b = (packed.astype(np.int64) & 0xFF).astype(np.uint8).reshape(D, F // 2)
        b = np.concatenate([b, np.zeros((D, (FP - F) // 2), np.uint8)], 1)
        am = absmax.reshape(D, F // blk).astype(np.float32)
        am = np.concatenate([am, np.zeros((D, (FP - F) // blk), np.float32)], 1)
        outs = []
        npairs = FS // 2
        nblk = FS // blk
        for i in range(cfg.ncores):
            bs = b[:, npairs * i:npairs * (i + 1)]       # [D, FS/2]
            ams = am[:, nblk * i:nblk * (i + 1)]         # [D, FS/blk]
            bT = np.ascontiguousarray(bs.T)              # [FS/2, D]
            amT = np.repeat(ams.T.astype(BF16), blk // 2, axis=0)  # [FS/2, D]
            outs.append((split_nib(bT), np.ascontiguousarray(amT)))
        return outs

    g = pack_rows(inputs["w_gate_packed"], inputs["w_gate_absmax"])
    u = pack_rows(inputs["w_up_packed"], inputs["w_up_absmax"])
    d = pack_down(inputs["w_down_packed"], inputs["w_down_absmax"])

    a_gu = np.concatenate(
        [inputs["w_gate_lora_a"], inputs["w_up_lora_a"]], axis=1
    )[dperm].astype(BF16)

    def pad_cols(m):
        return np.concatenate([m, np.zeros((m.shape[0], FP - F), m.dtype)], 1)

    b_g_full = pad_cols(inputs["w_gate_lora_b"].astype(np.float32))
    b_u_full = pad_cols(inputs["w_up_lora_b"].astype(np.float32))
    a_d_full = np.concatenate(
        [inputs["w_down_lora_a"].astype(np.float32),
         np.zeros((FP - F, R), np.float32)], 0
    )
    b_d = inputs["w_down_lora_b"].astype(BF16)

    in_maps = []
    for i in range(cfg.ncores):
        fsl = slice(FS * i, FS * (i + 1))
        in_maps.append({
            "xT": xT,
            "g_nib": g[i][0], "g_am": g[i][1],
            "u_nib": u[i][0], "u_am": u[i][1],
            "d_nib": d[i][0], "d_am": d[i][1],
            "a_gu": a_gu,
            "b_g": np.ascontiguousarray(b_g_full[:, fsl][:, fperm]).astype(BF16),
            "b_u": np.ascontiguousarray(b_u_full[:, fsl][:, fperm]).astype(BF16),
            "a_d": np.ascontiguousarray(a_d_full[fsl][fperm]).astype(BF16),
            "b_d": b_d,
            "ident": np.eye(128, dtype=BF16),
        })
    return in_maps


def _code_vals(inputs):
    # bf16-rounded codebook values as python floats
    return [float(v) for v in
            np.asarray(inputs["code"]).astype(BF16).astype(np.float32)]


def _assemble(cfg: Cfg, res):
    """Undo the ReduceScatter row interleave: core r's y_out row block c
    holds tokens [TRS*c + 128*r, +128)."""
    T, D = cfg.T, cfg.D
    y = np.empty((T, D), dtype=np.float32)
    for r in range(cfg.ncores):
        yr = res.results[r]["y_out"].astype(np.float32)
        for c in range(cfg.n_rs):
            t0 = cfg.TRS * c + 128 * r
            y[t0:t0 + 128] = yr[128 * c:128 * (c + 1)]
    return y


def run(cfg: Cfg, inputs, trace=False, **kwargs):
    code_vals = _code_vals(inputs)
    nc = _get_graph(cfg, code_vals)
    in_maps = _prep_inputs(cfg, inputs)
    res = run_bass_kernel_spmd(
        nc, in_maps, core_ids=list(range(cfg.ncores)), trace=trace, **kwargs
    )
    return _assemble(cfg, res), res


def kernel(**inputs) -> np.ndarray:
    cfg = Cfg()
    y, _ = run(cfg, inputs)
    return y.astype(np.float32)


# revision 8
# speedup vs baseline: 1.3352x; 1.1480x over previous
"""Trainium2 Bass kernel for nn_MixedGatedMLP (4-bit quantized gated MLP + LoRA).

v4: tensor-parallel over d_ff across 8 NeuronCores (F padded 11008->11264,
1408 rows/core).  The 16 codebook mask terms are merged on the PE array
(identity-matmul accumulation into PSUM, fp32) instead of DVE/GPSIMD adds.
 - codebook values baked as immediates (graph built after seeing inputs)
 - nibbles pre-split on host (u8 hi/lo planes); ACT converts u8->bf16;
   16 is_equal/mult mask terms on DVE; merge adds split DVE / dual-chain
   GPSIMD; blockwise absmax scale on DVE
 - down-proj dequant (to DRAM) interleaved with phase-1
 - phase-2 token-group-outer with all down weights resident, 8 PSUM banks,
   8 matmuls per LDWEIGHTS; ReduceScatter per 1024-token chunk, host
   reassembles the row interleave
"""

import sys

for _p in ("/opt/trn_rl_repo", "/root/.axon_site/_ro/trn_rl_repo"):
    if _p not in sys.path:
        sys.path.append(_p)

from contextlib import ExitStack

import numpy as np
import ml_dtypes

import concourse.bass as bass
import concourse.mybir as mybir
import concourse.tile as tile
from concourse import bacc
from concourse.bass_utils import run_bass_kernel_spmd

BF16 = ml_dtypes.bfloat16
NCORES = 8
ALU = mybir.AluOpType
AFT = mybir.ActivationFunctionType


class Cfg:
    def __init__(self, D=4096, T=4096, F=11008, R=16, block=64, ncores=8):
        self.D = D
        self.T = T
        self.F = F
        self.R = R
        self.block = block
        self.ncores = ncores
        unit = 2 * block * ncores
        self.FP = ((F + unit - 1) // unit) * unit   # 11264
        self.FS = self.FP // ncores                 # 1408
        self.TS = T // ncores                       # 512
        self.DP = D // 256                          # 16 byte-row chunks
        self.NT = T // 512                          # 8 token tiles
        self.f_slices = []
        f0 = 0
        while f0 < self.FS:
            w = min(512, self.FS - f0)
            self.f_slices.append((f0, w))
            f0 += w
        self.NTG = T // 128                         # 32 token groups
        self.NFG = self.FS // 128                   # 11 f groups
        self.ND = D // 512                          # 8 output-d slices
        self.TRS = 1024                             # tokens per ReduceScatter
        self.n_rs = T // self.TRS                   # 4
        self.j_chunks = []
        j0 = 0
        npairs = self.FS // 2
        while j0 < npairs:
            j1 = min(j0 + 128, npairs)
            self.j_chunks.append((j0, j1))
            j0 = j1
        # codebook terms whose merge adds run on gpsimd (two chains)
        self.gp_gu = 9
        self.gp_down = 9


def _dperm(D):
    """Row order of xT: per 256-d chunk, evens then odds."""
    idx = []
    for c in range(D // 256):
        base = 256 * c
        idx.extend(range(base, base + 256, 2))
        idx.extend(range(base + 1, base + 256, 2))
    return np.array(idx)


def _fperm_local(cfg):
    """Within-shard f order: per down j-chunk, even f (2j) then odd f (2j+1)."""
    idx = []
    for (j0, j1) in cfg.j_chunks:
        idx.extend(2 * j for j in range(j0, j1))
        idx.extend(2 * j + 1 for j in range(j0, j1))
    return np.array(idx)


def build_graph(cfg: Cfg, code_vals):
    """code_vals: 16 python floats (bf16-rounded codebook)."""
    nc = bacc.Bacc(None, num_devices=cfg.ncores)
    dt = mybir.dt
    D, T, FS, R = cfg.D, cfg.T, cfg.FS, cfg.R
    cv = [float(v) for v in code_vals]

    # ---- external inputs (per-core); nibble planes pre-split on host ----
    xT = nc.dram_tensor("xT", [D, T], dt.bfloat16, kind="ExternalInput")
    g_nib = nc.dram_tensor("g_nib", [D // 2, 2, FS], dt.uint8, kind="ExternalInput")
    u_nib = nc.dram_tensor("u_nib", [D // 2, 2, FS], dt.uint8, kind="ExternalInput")
    d_nib = nc.dram_tensor("d_nib", [FS // 2, 2, D], dt.uint8, kind="ExternalInput")
    g_am = nc.dram_tensor("g_am", [D // 2, FS], dt.bfloat16, kind="ExternalInput")
    u_am = nc.dram_tensor("u_am", [D // 2, FS], dt.bfloat16, kind="ExternalInput")
    d_am = nc.dram_tensor("d_am", [FS // 2, D], dt.bfloat16, kind="ExternalInput")
    a_gu = nc.dram_tensor("a_gu", [D, 2 * R], dt.bfloat16, kind="ExternalInput")
    b_g = nc.dram_tensor("b_g", [R, FS], dt.bfloat16, kind="ExternalInput")
    b_u = nc.dram_tensor("b_u", [R, FS], dt.bfloat16, kind="ExternalInput")
    a_d = nc.dram_tensor("a_d", [FS, R], dt.bfloat16, kind="ExternalInput")
    b_d = nc.dram_tensor("b_d", [R, D], dt.bfloat16, kind="ExternalInput")
    ident = nc.dram_tensor("ident", [128, 128], dt.bfloat16, kind="ExternalInput")

    # y_out rows: 4 blocks of 128 tokens (one per ReduceScatter chunk)
    y_out = nc.dram_tensor("y_out", [cfg.TS, D], dt.bfloat16, kind="ExternalOutput")

    # ---- internal DRAM ----
    x3_dram = nc.dram_tensor("x3_dram", [FS, T], dt.bfloat16, kind="Internal")
    xag_dram = nc.dram_tensor("xag_dram", [R, T], dt.bfloat16, kind="Internal")
    xau_dram = nc.dram_tensor("xau_dram", [R, T], dt.bfloat16, kind="Internal")
    x3a_dram = nc.dram_tensor("x3a_dram", [R, T], dt.bfloat16, kind="Internal")
    wd_dram = nc.dram_tensor("wd_dram", [FS, D], dt.bfloat16, kind="Internal")
    rs_in = [
        nc.dram_tensor(f"rs_in{c}", [cfg.TRS, D], dt.bfloat16, kind="Internal")
        for c in range(cfg.n_rs)
    ]
    rs_out = [
        nc.dram_tensor(f"rs_out{c}", [cfg.TRS // cfg.ncores, D], dt.bfloat16,
                       kind="Internal")
        for c in range(cfg.n_rs)
    ]

    rg = [list(range(cfg.ncores))]

    with tile.TileContext(nc) as tc, ExitStack() as ctx:
        const_pool = ctx.enter_context(tc.tile_pool(name="const", bufs=1))
        agu_sb = const_pool.tile([128, D // 128, 2 * R], dt.bfloat16)
        nc.sync.dma_start(agu_sb[:], a_gu.rearrange("(c p) r -> p c r", p=128))
        bg_sb = const_pool.tile([R, FS], dt.bfloat16)
        nc.sync.dma_start(bg_sb[:], b_g[:])
        bu_sb = const_pool.tile([R, FS], dt.bfloat16)
        nc.sync.dma_start(bu_sb[:], b_u[:])
        ad_sb = const_pool.tile([128, FS // 128, R], dt.bfloat16)
        nc.sync.dma_start(ad_sb[:], a_d.rearrange("(c p) r -> p c r", p=128))
        bd_sb = const_pool.tile([R, D], dt.bfloat16)
        nc.sync.dma_start(bd_sb[:], b_d[:])
        id_sb = const_pool.tile([128, 128], dt.bfloat16)
        nc.sync.dma_start(id_sb[:], ident[:])

        dq_pool = ctx.enter_context(tc.tile_pool(name="dq", bufs=3))
        tg_pool = ctx.enter_context(tc.tile_pool(name="tgp", bufs=4))
        tk_pool = ctx.enter_context(tc.tile_pool(name="tkp", bufs=4))
        wdd_pool = ctx.enter_context(tc.tile_pool(name="wdd", bufs=3))
        psm_pool = []

        def dequant_pair(nib_dram, am_dram, p0, pc, f0, fw, W, gp_terms,
                         planes=(0, 1), pbase=0):
            """Dequant nibble rows [p0:p0+pc] x cols [f0:f0+fw] into W
            (tile [128, 2, fw] bf16): W[:, 0, :] = hi plane, W[:, 1, :] = lo.
            The 16 mask terms are accumulated on the PE array (identity
            matmul) in fp32 PSUM.  Ops use partitions [pbase, pbase+pc)."""
            psl = slice(pbase, pbase + pc)
            U = dq_pool.tile([128, 2, fw], dt.uint8, tag="u")
            S2 = dq_pool.tile([128, 2, fw], dt.bfloat16, tag="sq")
            for pl in planes:
                nc.sync.dma_start(U[psl, pl],
                                  nib_dram[p0:p0 + pc, pl, f0:f0 + fw])
                nc.sync.dma_start(S2[psl, pl],
                                  am_dram[p0:p0 + pc, f0:f0 + fw])
            X = dq_pool.tile([128, 2, fw], dt.bfloat16, tag="x")
            if len(planes) == 2:
                s_ = (slice(None), slice(None))
            else:
                s_ = (planes[0], slice(None))
            nc.scalar.copy(X[(psl, *s_)], U[(psl, *s_)])
            Xs = X[(psl, *s_)]

            pm = psm_pool[0].tile([128, 2, 512], dt.float32, tag="pm",
                                 name="pm")
            for k in range(16):
                pool = tk_pool if k % 2 == 0 else tg_pool
                tk = pool.tile([128, 2, fw], dt.bfloat16, tag="tk",
                               name=f"tk{k}")
                nc.vector.tensor_scalar(tk[(psl, *s_)], Xs, float(k), cv[k],
                                        ALU.is_equal, ALU.mult)
                for pl in planes:
                    nc.tensor.matmul(pm[psl, pl, 0:fw], id_sb[psl, psl],
                                     tk[psl, pl],
                                     start=(k == 0), stop=(k == 15))
            for pl in planes:
                nc.vector.tensor_tensor(W[psl, pl], pm[psl, pl, 0:fw],
                                        S2[psl, pl], ALU.mult)

        # down-proj dequant jobs (interleaved into phase 1), 512-wide
        down_jobs = []
        for dh in range(cfg.ND):
            dd0 = 512 * dh
            for ic, (j0, j1) in enumerate(cfg.j_chunks):
                down_jobs.append((ic, j0, j1 - j0, dd0))

        def emit_down(job):
            ic, j0, pc, dd0 = job
            Wt = wdd_pool.tile([128, 2, 512], dt.bfloat16, tag="wdd")
            if pc == 128:
                dequant_pair(d_nib, d_am, j0, pc, dd0, 512, Wt, cfg.gp_down)
                nc.sync.dma_start(
                    wd_dram[256 * ic:256 * ic + 128, dd0:dd0 + 512], Wt[:, 0])
                nc.sync.dma_start(
                    wd_dram[256 * ic + 128:256 * ic + 256, dd0:dd0 + 512],
                    Wt[:, 1])
            else:
                dequant_pair(d_nib, d_am, j0, pc, dd0, 512, Wt, cfg.gp_down,
                             planes=(0,), pbase=0)
                dequant_pair(d_nib, d_am, j0, pc, dd0, 512, Wt, cfg.gp_down,
                             planes=(1,), pbase=pc)
                nc.sync.dma_start(
                    wd_dram[256 * ic:256 * ic + pc, dd0:dd0 + 512],
                    Wt[0:pc, 0])
                nc.sync.dma_start(
                    wd_dram[256 * ic + pc:256 * ic + 2 * pc, dd0:dd0 + 512],
                    Wt[pc:2 * pc, 1])

        n_dj = len(down_jobs)
        dj_pos = 0

        # =============== phase 1: gate/up matmuls -> x3 ===============
        XQ = 4                       # xT chunk-quarters per token tile
        with (
            tc.tile_pool(name="w", bufs=cfg.DP + 2) as w_pool,
            tc.tile_pool(name="xt", bufs=XQ + 1) as xt_pool,
            tc.tile_pool(name="p1", bufs=3) as p1_pool,
            tc.tile_pool(name="psgu", bufs=2, space="PSUM") as ps_gu,
            tc.tile_pool(name="pslora", bufs=1, space="PSUM") as ps_lora,
            tc.tile_pool(name="psm", bufs=1, space="PSUM") as psm_pool_,
        ):
            psm_pool.append(psm_pool_)
            for si, (f0, fw) in enumerate(cfg.f_slices):
                wg = []
                wu = []
                for c in range(cfg.DP):
                    for name, bsrc, asrc, wlist in (
                        ("g", g_nib, g_am, wg), ("u", u_nib, u_am, wu),
                    ):
                        Wt = w_pool.tile([128, 2, fw], dt.bfloat16,
                                         tag=f"w{name}")
                        dequant_pair(bsrc, asrc, 128 * c, 128, f0, fw, Wt,
                                     cfg.gp_gu)
                        wlist.append(Wt)
                    if dj_pos < n_dj:
                        emit_down(down_jobs[dj_pos])
                        dj_pos += 1
                for t in range(cfg.NT):
                    tt = slice(512 * t, 512 * (t + 1))
                    nxq = 2 * cfg.DP // XQ
                    xts = []
                    for quar in range(XQ):
                        xt_t = xt_pool.tile([128, nxq, 512], dt.bfloat16,
                                            tag="xt")
                        for ci in range(nxq):
                            cc = nxq * quar + ci
                            nc.sync.dma_start(
                                xt_t[:, ci, :],
                                xT[128 * cc:128 * (cc + 1), tt])
                        xts.append(xt_t)

                    def xchunk(ci):
                        return xts[ci // nxq][:, ci % nxq, :]

                    if si == 0:
                        for ri, dst in ((0, xag_dram), (1, xau_dram)):
                            pa = ps_lora.tile([R, 512], dt.float32, tag="pa")
                            for ci in range(2 * cfg.DP):
                                nc.tensor.matmul(
                                    pa[:], agu_sb[:, ci, R * ri:R * (ri + 1)],
                                    xchunk(ci),
                                    start=(ci == 0),
                                    stop=(ci == 2 * cfg.DP - 1))
                            with tc.high_priority():
                                st = p1_pool.tile([R, 512], dt.bfloat16,
                                                  tag="st")
                                nc.scalar.copy(st[:], pa[:])
                                nc.sync.dma_start(dst[:, tt], st[:])
                    xag_t = p1_pool.tile([R, 512], dt.bfloat16, tag="xag_t")
                    nc.sync.dma_start(xag_t[:], xag_dram[:, tt])
                    xau_t = p1_pool.tile([R, 512], dt.bfloat16, tag="xau_t")
                    nc.sync.dma_start(xau_t[:], xau_dram[:, tt])
                    for g in range(fw // 128):
                        fg = slice(128 * g, 128 * (g + 1))
                        fga = slice(f0 + 128 * g, f0 + 128 * (g + 1))
                        pg = ps_gu.tile([128, 512], dt.float32, tag="pg")
                        pu = ps_gu.tile([128, 512], dt.float32, tag="pu")
                        for ci in range(2 * cfg.DP):
                            nc.tensor.matmul(pg[:], wg[ci // 2][:, ci % 2, fg],
                                             xchunk(ci),
                                             start=(ci == 0), stop=False)
                        nc.tensor.matmul(pg[:], bg_sb[:, fga], xag_t[:],
                                         start=False, stop=True)
                        for ci in range(2 * cfg.DP):
                            nc.tensor.matmul(pu[:], wu[ci // 2][:, ci % 2, fg],
                                             xchunk(ci),
                                             start=(ci == 0), stop=False)
                        nc.tensor.matmul(pu[:], bu_sb[:, fga], xau_t[:],
                                         start=False, stop=True)
                        with tc.high_priority():
                            sg = p1_pool.tile([128, 512], dt.bfloat16,
                                              tag="sg")
                            nc.scalar.activation(sg[:], pg[:], AFT.Silu)
                            pu_sb = p1_pool.tile([128, 512], dt.bfloat16,
                                                 tag="pusb")
                            nc.scalar.copy(pu_sb[:], pu[:])
                            x3t = p1_pool.tile([128, 512], dt.bfloat16,
                                               tag="x3t")
                            nc.vector.tensor_tensor(x3t[:], sg[:], pu_sb[:],
                                                    ALU.mult)
                            nc.sync.dma_start(x3_dram[fga, tt], x3t[:])
            while dj_pos < n_dj:
                emit_down(down_jobs[dj_pos])
                dj_pos += 1

        # =============== phase 2a: x3 @ Ad -> x3a ===============
        with (
            tc.tile_pool(name="p2a", bufs=4) as p2a_pool,
            tc.tile_pool(name="psda", bufs=1, space="PSUM") as ps_da,
        ):
            for t in range(cfg.NT):
                tt = slice(512 * t, 512 * (t + 1))
                pa = ps_da.tile([R, 512], dt.float32, tag="pa2")
                for g in range(cfg.NFG):
                    x3l = p2a_pool.tile([128, 512], dt.bfloat16, tag="x3a_in")
                    nc.sync.dma_start(x3l[:],
                                      x3_dram[128 * g:128 * (g + 1), tt])
                    nc.tensor.matmul(pa[:], ad_sb[:, g, :], x3l[:],
                                     start=(g == 0), stop=(g == cfg.NFG - 1))
                st2 = p2a_pool.tile([R, 512], dt.bfloat16, tag="st2")
                nc.scalar.copy(st2[:], pa[:])
                nc.sync.dma_start(x3a_dram[:, tt], st2[:])

        # ======== phase 2b: down matmul (tg-outer) + ReduceScatter ========
        with (
            tc.tile_pool(name="wdr", bufs=1) as wdr_pool,
            tc.tile_pool(name="x3p", bufs=2 * cfg.NFG + 2) as x3p_pool,
            tc.tile_pool(name="p2", bufs=6) as p2_pool,
            tc.tile_pool(name="psd", bufs=1, space="PSUM") as ps_d,
        ):
            wdq = []
            for g in range(cfg.NFG):
                wt = wdr_pool.tile([128, D], dt.bfloat16, tag=f"wdr{g}",
                                   name=f"wdr{g}")
                nc.sync.dma_start(wt[:], wd_dram[128 * g:128 * (g + 1), :])
                wdq.append(wt)
            for tg in range(cfg.NTG):
                tsl = slice(128 * tg, 128 * (tg + 1))
                rs_c = tg // 8
                row0 = 128 * (tg % 8)
                x3at = p2_pool.tile([R, 128], dt.bfloat16, tag="x3at")
                nc.sync.dma_start(x3at[:], x3a_dram[:, tsl])
                x3ls = []
                for g in range(cfg.NFG):
                    x3l = x3p_pool.tile([128, 128], dt.bfloat16, tag="x3l")
                    nc.sync.dma_start(x3l[:],
                                      x3_dram[128 * g:128 * (g + 1), tsl])
                    x3ls.append(x3l)
                for half in range(2):
                    dhs = range(4 * half, 4 * half + 4)
                    pds = {dh: ps_d.tile([128, 512], dt.float32,
                                         tag=f"pd{dh}",
                                         name=f"pd{tg}_{dh}")
                           for dh in dhs}
                    for g in range(cfg.NFG):
                        for dh in dhs:
                            nc.tensor.matmul(
                                pds[dh][:], x3ls[g][:],
                                wdq[g][:, 512 * dh:512 * (dh + 1)],
                                start=(g == 0), stop=False)
                    for dh in dhs:
                        nc.tensor.matmul(
                            pds[dh][:], x3at[:],
                            bd_sb[:, 512 * dh:512 * (dh + 1)],
                            start=False, stop=True)
                    with tc.high_priority():
                        for dh in dhs:
                            yb = p2_pool.tile([128, 512], dt.bfloat16,
                                              tag="yb")
                            nc.scalar.copy(yb[:], pds[dh][:])
                            nc.sync.dma_start(
                                rs_in[rs_c][row0:row0 + 128,
                                            512 * dh:512 * (dh + 1)], yb[:])
                if tg % 8 == 7:
                    nc.gpsimd.collective_compute(
                        "ReduceScatter", ALU.add, replica_groups=rg,
                        ins=[rs_in[rs_c][:, :].opt()],
                        outs=[rs_out[rs_c][:, :].opt()],
                    )
                    nc.sync.dma_start(
                        y_out[128 * rs_c:128 * (rs_c + 1), :],
                        rs_out[rs_c][:, :])

    nc.compile()
    return nc


# ----------------- host side -----------------

_CACHE = {}


def _get_graph(cfg: Cfg, code_vals):
    key = (cfg.D, cfg.T, cfg.F, cfg.ncores, tuple(code_vals))
    if key not in _CACHE:
        _CACHE[key] = build_graph(cfg, code_vals)
    return _CACHE[key]


def _prep_inputs(cfg: Cfg, inputs):
    """Shard + lay out the full inputs for each core. Marshalling only."""
    D, T, F, FP, FS, R = cfg.D, cfg.T, cfg.F, cfg.FP, cfg.FS, cfg.R
    blk = cfg.block
    dperm = _dperm(D)
    fperm = _fperm_local(cfg)

    x = inputs["x"]
    xT = np.ascontiguousarray(x.T[dperm]).astype(BF16)

    def split_nib(bT):
        """bytes [P, Q] -> [P, 2, Q] u8 nibble planes (hi, lo)."""
        return np.ascontiguousarray(
            np.stack([bT >> 4, bT & 15], axis=1))

    def pack_rows(packed, absmax):
        """gate/up: packed [F*D/2] -> per-core (nib [D/2, 2, FS], am)."""
        b = (packed.astype(np.int64) & 0xFF).astype(np.uint8).reshape(F, D // 2)
        b = np.concatenate([b, np.zeros((FP - F, D // 2), np.uint8)], 0)
        am = absmax.reshape(F, D // blk).astype(np.float32)
        am = np.concatenate([am, np.zeros((FP - F, D // blk), np.float32)], 0)
        outs = []
        for i in range(cfg.ncores):
            bs = b[FS * i:FS * (i + 1)][fperm]           # [FS, D/2]
            ams = am[FS * i:FS * (i + 1)][fperm]         # [FS, D/blk]
            bT = np.ascontiguousarray(bs.T)              # [D/2, FS]
            amT = np.repeat(ams.T.astype(BF16), blk // 2, axis=0)  # [D/2, FS]
            outs.append((split_nib(bT), np.ascontiguousarray(amT)))
        return outs

    def pack_down(packed, absmax):
        """down: packed [D*F/2] -> per-core (nib [FS/2, 2, D], am [FS/2, D])."""
        b = (packed.astype(np.int64) & 0xFF).astype(np.uint8).reshape(D, F // 2)
        b = np.concatenate([b, np.zeros((D, (FP - F) // 2), np.uint8)], 1)
        am = absmax.reshape(D, F // blk).astype(np.float32)
        am = np.concatenate([am, np.zeros((D, (FP - F) // blk), np.float32)], 1)
        outs = []
        npairs = FS // 2
        nblk = FS // blk
        for i in range(cfg.ncores):
            bs = b[:, npairs * i:npairs * (i + 1)]       # [D, FS/2]
            ams = am[:, nblk * i:nblk * (i + 1)]         # [D, FS/blk]
            bT = np.ascontiguousarray(bs.T)              # [FS/2, D]
            amT = np.repeat(ams.T.astype(BF16), blk // 2, axis=0)  # [FS/2, D]
            outs.append((split_nib(bT), np.ascontiguousarray(amT)))
        return outs

    g = pack_rows(inputs["w_gate_packed"], inputs["w_gate_absmax"])
    u = pack_rows(inputs["w_up_packed"], inputs["w_up_absmax"])
    d = pack_down(inputs["w_down_packed"], inputs["w_down_absmax"])

    a_gu = np.concatenate(
        [inputs["w_gate_lora_a"], inputs["w_up_lora_a"]], axis=1
    )[dperm].astype(BF16)

    def pad_cols(m):
        return np.concatenate([m, np.zeros((m.shape[0], FP - F), m.dtype)], 1)

    b_g_full = pad_cols(inputs["w_gate_lora_b"].astype(np.float32))
    b_u_full = pad_cols(inputs["w_up_lora_b"].astype(np.float32))
    a_d_full = np.concatenate(
        [inputs["w_down_lora_a"].astype(np.float32),
         np.zeros((FP - F, R), np.float32)], 0
    )
    b_d = inputs["w_down_lora_b"].astype(BF16)

    in_maps = []
    for i in range(cfg.ncores):
        fsl = slice(FS * i, FS * (i + 1))
        in_maps.append({
            "xT": xT,
            "g_nib": g[i][0], "g_am": g[i][1],
            "u_nib": u[i][0], "u_am": u[i][1],
            "d_nib": d[i][0], "d_am": d[i][1],
            "a_gu": a_gu,
            "b_g": np.ascontiguousarray(b_g_full[:, fsl][:, fperm]).astype(BF16),
            "b_u": np.ascontiguousarray(b_u_full[:, fsl][:, fperm]).astype(BF16),
            "a_d": np.ascontiguousarray(a_d_full[fsl][fperm]).astype(BF16),
            "b_d": b_d,
            "ident": np.eye(128, dtype=BF16),
        })
    return in_maps


def _code_vals(inputs):
    # bf16-rounded codebook values as python floats
    return [float(v) for v in
            np.asarray(inputs["code"]).astype(BF16).astype(np.float32)]


def _assemble(cfg: Cfg, res):
    """Undo the ReduceScatter row interleave: core r's y_out row block c
    holds tokens [TRS*c + 128*r, +128)."""
    T, D = cfg.T, cfg.D
    y = np.empty((T, D), dtype=np.float32)
    for r in range(cfg.ncores):
        yr = res.results[r]["y_out"].astype(np.float32)
        for c in range(cfg.n_rs):
            t0 = cfg.TRS * c + 128 * r
            y[t0:t0 + 128] = yr[128 * c:128 * (c + 1)]
    return y


def run(cfg: Cfg, inputs, trace=False, **kwargs):
    code_vals = _code_vals(inputs)
    nc = _get_graph(cfg, code_vals)
    in_maps = _prep_inputs(cfg, inputs)
    res = run_bass_kernel_spmd(
        nc, in_maps, core_ids=list(range(cfg.ncores)), trace=trace, **kwargs
    )
    return _assemble(cfg, res), res


def kernel(**inputs) -> np.ndarray:
    cfg = Cfg()
    y, _ = run(cfg, inputs)
    return y.astype(np.float32)
